# revision 4
# baseline (speedup 1.0000x reference)
"""Causal Conv1d (B=8, C=256, T=4096, H=512, K=4) on 8 TRN2 NeuronCores.

Strategy: data-parallel over batch — core i computes batch i.
Per core: out[h, t] = sum_{k, c} W[h, c*K+k] * xpad[c, t+k] + bias[h]
where xpad is x left-padded by K-1 zeros (host side).

Weight-stationary schedule (measured on HW via probe.py): a fresh
stationary operand per matmul costs ~297 ns per 512-col MM (serialized
LDWEIGHTS), while chains that reuse the loaded weights across several
accumulating matmuls into different PSUM banks issue at ~190 ns/MM
(0.371 ns/col). So instead of the im2col order (t-tile outer, weight
chunk inner = LDW every MM), the loop is weight-chunk outer:

  for tg in {0,1}:            # halves of the 8 t-tiles
    for hj in 4:              # output-channel chunk (PSUM partition dim)
      for q in 8:             # (tap k, c-chunk) weight chunk = LDW
        for tj in 4:          # t-tiles in the half: 4 MMs reuse the LDW
          ps[tj] += wt[hj,q].T @ x[cc][:, ti*512+k : +512]

Groups of 4 psum banks alternate 0-3/4-7, so the bias-add+copy-out of
one group (split across DVE and ACT) fully overlaps the next group's
matmuls. x is SBUF-resident as 4 disjoint tiles (2 c-chunks x 2
t-halves); the tg-outer order gives each tile a half-body idle window
so the next rep's x/W DMA reload pipelines behind compute in the
timing loop. Inputs stream as float32r (tf32, host-pre-rounded, full
PE rate); accumulation is fp32 in PSUM.

Roofline accounting per core: 256 MMs x ~190 ns = 48.6 us PE (the
measured issue-rate floor for this stack), DMA 14.6 MB at 358 GB/s =
41 us (hidden), DVE/ACT copies 2M els (hidden). Rel err vs fp32
reference: ~2.9e-4.
"""

import numpy as np

import concourse.bass as bass
import concourse.mybir as mybir
import concourse.tile as tile
from concourse import bacc
from concourse import bass2jax

B, C, T = 8, 256, 4096
H, K = 512, 4
PAD = K - 1

N_CORES = 8
TT = 512                # t-tile (free dim per matmul, one fp32 PSUM bank)
N_TTILES = T // TT      # 8
N_HCHUNK = H // 128     # 4
N_CCHUNK = C // 128     # 2
N_MM = N_CCHUNK * K     # 8 weight chunks per (hj) output chunk
HALF = N_TTILES // 2    # t-tiles per tg half
XCOLS = HALF * TT + PAD  # 2051 cols per x half-tile

_COMPILED = {}


def _build(reps=1, hint=False, staggered=True):
    f32 = mybir.dt.float32
    f32r = mybir.dt.float32r
    nc = bacc.Bacc("TRN2", target_bir_lowering=False, debug=False)

    # x/wt hold host-side tf32-rounded data; declaring them fp32r lets the
    # matmul consume DMA'd tiles directly (no on-chip rounding pass).
    # x is staged host-side as 4 disjoint half tiles [cc, tg] so the SBUF
    # tiles share no DMA region (clean cross-rep pipelining deps).
    x_ext = nc.declare_dram_parameter(
        "x", [N_CCHUNK * 2, 128, XCOLS], f32r, isOutput=False
    )
    # wt[hj][c, q*128+m]: lhsT for (q=k*N_CCHUNK+cc, h-chunk hj).
    wt_ext = nc.declare_dram_parameter(
        "wt", [N_HCHUNK, 128, N_MM * 128], f32r, isOutput=False
    )
    # bias_mat[p, j] = b[j*128 + p]
    b_ext = nc.declare_dram_parameter("bias", [128, N_HCHUNK], f32, isOutput=False)
    out_ext = nc.declare_dram_parameter("out", [H, T], f32, isOutput=True)

    CH = N_MM * 128  # per-h-chunk weight columns

    with tile.TileContext(nc) as tc:
        with (
            tc.tile_pool(name="wpool", bufs=1) as wpool,
            tc.tile_pool(name="opool", bufs=8) as opool,
            tc.tile_pool(name="psum", bufs=8, space="PSUM") as psum_pool,
        ):

            def body():
                # Resident x: 4 tiles [128, 2051], one DMA each.
                xts = {}
                for cc in range(N_CCHUNK):
                    for tg in range(2):
                        xt = wpool.tile([128, XCOLS], f32r, name=f"x{cc}{tg}")
                        nc.sync.dma_start(xt[:], x_ext[cc * 2 + tg])
                        xts[cc, tg] = xt
                # Resident W, chunked by hj so reload deps are per-chunk.
                wtile = wpool.tile([128, N_HCHUNK * CH], f32r, name="wtile")
                for hj in range(N_HCHUNK):
                    nc.sync.dma_start(wtile[:, hj * CH : (hj + 1) * CH], wt_ext[hj])
                btile = wpool.tile([128, N_HCHUNK], f32, name="btile")
                nc.sync.dma_start(btile[:], b_ext[:])

                for tg in range(2):
                    for hj in range(N_HCHUNK):
                        pss = [
                            psum_pool.tile([128, TT], f32, name="ps", tag="ps")
                            for _ in range(HALF)
                        ]
                        for q in range(N_MM):
                            k, cc = divmod(q, N_CCHUNK)
                            for tj in range(HALF):
                                nc.tensor.matmul(
                                    pss[tj][:],
                                    wtile[
                                        :,
                                        hj * CH + q * 128 : hj * CH + q * 128 + 128,
                                    ],
                                    xts[cc, tg][:, tj * TT + k : tj * TT + k + TT],
                                    start=(q == 0),
                                    stop=(q == N_MM - 1),
                                )
                        for tj in range(HALF):
                            ti = tg * HALF + tj
                            ot = opool.tile([128, TT], f32, name="ot", tag="ot")
                            if tj % 2:
                                nc.scalar.add(ot[:], pss[tj][:], btile[:, hj : hj + 1])
                            else:
                                nc.vector.tensor_scalar_add(
                                    ot[:], pss[tj][:], btile[:, hj : hj + 1]
                                )
                            nc.sync.dma_start(
                                out_ext[
                                    hj * 128 : (hj + 1) * 128,
                                    ti * TT : (ti + 1) * TT,
                                ],
                                ot[:],
                            )

            if reps == 1:
                body()
            else:
                kw = {"staggered_reset": staggered}
                if hint:
                    kw["hint_engines"] = (mybir.EngineType.PE,)
                with tc.For_i(0, reps, 1, **kw):
                    body()

    nc.compile()
    return nc


def get_nc():
    if "nc" not in _COMPILED:
        _COMPILED["nc"] = _build()
    return _COMPILED["nc"]


def _tf32_round(a):
    """Round fp32 to tf32 (10-bit mantissa) with round-to-nearest-even."""
    u = np.ascontiguousarray(a, dtype=np.float32).view(np.uint32)
    lsb = (u >> np.uint32(13)) & np.uint32(1)
    u = u + np.uint32(0x0FFF) + lsb
    u &= np.uint32(0xFFFFE000)
    return u.view(np.float32)


def _prep_inputs(x, W, b):
    x = _tf32_round(np.asarray(x, dtype=np.float32))
    W = _tf32_round(np.asarray(W, dtype=np.float32))
    b = np.asarray(b, dtype=np.float32)

    xpad = np.zeros((B, C, T + PAD), dtype=np.float32)
    xpad[:, :, PAD:] = x

    # 4 disjoint half tiles per batch: [cc*2+tg] -> xpad cols
    # [tg*HALF*TT : tg*HALF*TT + XCOLS] of channel chunk cc.
    xh = np.empty((B, N_CCHUNK * 2, 128, XCOLS), dtype=np.float32)
    for cc in range(N_CCHUNK):
        for tg in range(2):
            c0 = tg * HALF * TT
            xh[:, cc * 2 + tg] = xpad[
                :, cc * 128 : (cc + 1) * 128, c0 : c0 + XCOLS
            ]

    kern = W.reshape(H, C, K)
    wt = np.empty((N_HCHUNK, 128, N_MM * 128), dtype=np.float32)
    for hj in range(N_HCHUNK):
        for k in range(K):
            for cc in range(N_CCHUNK):
                q = k * N_CCHUNK + cc
                wt[hj, :, q * 128 : (q + 1) * 128] = kern[
                    hj * 128 : (hj + 1) * 128, cc * 128 : (cc + 1) * 128, k
                ].T

    bias_mat = np.ascontiguousarray(b.reshape(N_HCHUNK, 128).T)
    return xh, wt, bias_mat


def _get_exec():
    """Build (once) a jitted shard_map executable over the 8 cores.

    Mirrors bass2jax.run_bass_via_pjrt but caches the compiled callable so
    repeated runs (timing loops) don't re-trace / re-compile.
    """
    if "exec" in _COMPILED:
        return _COMPILED["exec"]

    import jax
    from jax.experimental.shard_map import shard_map
    from jax.sharding import Mesh, PartitionSpec

    nc = get_nc()
    bass2jax.install_neuronx_cc_hook()
    assert nc.dbg_addr is None
    partition_name = nc.partition_id_tensor.name if nc.partition_id_tensor else None

    in_names, out_names, out_avals, zero_outs = [], [], [], []
    for alloc in nc.m.functions[0].allocations:
        if not isinstance(alloc, mybir.MemoryLocationSet):
            continue
        name = alloc.memorylocations[0].name
        if alloc.kind == "ExternalInput":
            if name != partition_name:
                in_names.append(name)
        elif alloc.kind == "ExternalOutput":
            shape = tuple(alloc.tensor_shape)
            dtype = mybir.dt.np(alloc.dtype)
            out_names.append(name)
            out_avals.append(jax.core.ShapedArray(shape, dtype))
            zero_outs.append(np.zeros(shape, dtype))
    n_params = len(in_names)
    all_names = in_names + out_names
    if partition_name is not None:
        all_names = all_names + [partition_name]

    def _body(*args):
        operands = list(args)
        if partition_name is not None:
            operands.append(bass2jax.partition_id_tensor())
        outs = bass2jax._bass_exec_p.bind(
            *operands,
            out_avals=tuple(out_avals),
            in_names=tuple(all_names),
            out_names=tuple(out_names),
            lowering_input_output_aliases=(),
            sim_require_finite=True,
            sim_require_nnan=True,
            nc=nc,
        )
        return tuple(outs)

    devices = jax.devices()[:N_CORES]
    mesh = Mesh(np.asarray(devices), ("core",))
    n_args = n_params + len(out_names)
    sharded = jax.jit(
        shard_map(
            _body,
            mesh=mesh,
            in_specs=(PartitionSpec("core"),) * n_args,
            out_specs=(PartitionSpec("core"),) * len(out_names),
            check_rep=False,
        ),
        keep_unused=True,
    )
    _COMPILED["exec"] = (sharded, in_names, out_names, out_avals, zero_outs, mesh)
    return _COMPILED["exec"]


def _make_args(in_maps):
    sharded, in_names, out_names, out_avals, zero_outs, mesh = _get_exec()
    concat_in = [
        np.concatenate([np.asarray(in_maps[c][nm]) for c in range(N_CORES)], axis=0)
        for nm in in_names
    ]
    concat_zeros = [
        np.zeros((N_CORES * z.shape[0], *z.shape[1:]), z.dtype) for z in zero_outs
    ]
    return concat_in + concat_zeros


def _run(in_maps):
    sharded, in_names, out_names, out_avals, zero_outs, mesh = _get_exec()
    out_arrs = sharded(*_make_args(in_maps))
    return [
        {
            nm: np.asarray(out_arrs[i]).reshape(N_CORES, *out_avals[i].shape)[c]
            for i, nm in enumerate(out_names)
        }
        for c in range(N_CORES)
    ]


def make_in_maps(x, W, b):
    xh, wt, bias_mat = _prep_inputs(x, W, b)
    return [
        {"x": np.ascontiguousarray(xh[i]), "wt": wt, "bias": bias_mat}
        for i in range(N_CORES)
    ]


def kernel(x, W, b):
    results = _run(make_in_maps(x, W, b))
    return np.stack([results[i]["out"] for i in range(N_CORES)], axis=0)
